# revision 80
# baseline (speedup 1.0000x reference)
"""Answer-pointer network forward pass on 8 TRN2 NeuronCores.

Data-parallel over batch: B=64 -> 8 batches per core, weights replicated.
No collectives; each core computes softmax attention maps (aP1, aP2) for
its batch shard and the host concatenates.

Layouts (host-side prep, outside HW exec):
  - peC  [512,8,2048]-ish fp16: passEnc feature-major for the WPh linear
    (contract over d=512 on partitions).
  - peN8 [8pair,128,8b,2sub,512] fp8e4: passEnc position-major for the
    attention-weighted context ct, pre-packed for fp8 DoubleRow matmuls
    (contract over p=2048; each DR instruction consumes K=256 = 2 subtiles
    of 128 partitions at the cost of one 512-column stream).

Measured PE cost law (hardware microbench): a matmul instruction costs
N_out_columns x 0.417ns regardless of M, K-chaining, or ldweights; fp8
DoubleRow processes K=256 per instruction at the same cost, i.e. exactly
2x fp16 throughput. Hence ct (contract 2048) uses DR-fp8 (64 instrs
instead of 128), while passP/sP stay fp16 (fp8 would need hi/lo splits
that cost more instructions than fp16).

Schedule: the Q phase (question-aware init state) is a long serial
cross-engine chain; it is emitted interleaved with the first passP
chunk's matmuls so its dependency stalls hide under the initial peC DMA
wait. The GRU consumes the *unnormalized* context sum (matmul is linear)
so its matmuls start before the softmax normalizer 1/Z is ready; 1/Z is
folded into the gate bias-add (scalar_tensor_tensor). aP1 normalization
and store run during P2, off the critical path.

Per-batch reductions (sP, sQ, rQ, ct) use masked stationary operands:
column b of the lhsT kept, rest zeroed, so batch b's matmul writes only
PSUM row b; accumulating over b assembles [8, N] without partition-offset
copies.
"""

import numpy as np

try:
    import concourse.bass as bass
except ImportError:  # pragma: no cover
    import sys

    sys.path.insert(0, "/opt/trn_rl_repo")
    import concourse.bass as bass

import concourse.tile as tile
from concourse import bacc, mybir
from concourse.bass_utils import run_bass_kernel_spmd

F8 = mybir.dt.float8e4
F16 = mybir.dt.float16
F32 = mybir.dt.float32
AF = mybir.ActivationFunctionType
OP = mybir.AluOpType
DR = mybir.MatmulPerfMode.DoubleRow

H = 256      # hidden
D = 512      # 2*hidden
LP = 2048    # passage length
LQ = 64      # question length
B = 64       # global batch
BL = 8       # batch per core
G = 6 * H    # 1536, GRU gate width
NC = 8       # cores
NKT = D // 128    # 4 contraction tiles over d
NHT = H // 128    # 2 tiles over h
NPC = LP // 512   # 4 p-chunks of 512
NPR = LP // 256   # 8 p-pairs of 256 (DoubleRow K tiles)


def _layout(entries):
    off, table = 0, {}
    for name, ln in entries:
        table[name] = (off, ln)
        off += ln
    return table, off


WA, WATOT = _layout([
    ("WQvT", NHT * H), ("WQuT", NHT * 2 * H), ("WPhT", NKT * H), ("WahT", NKT * H),
    ("VQrT", NHT), ("Vt1T", NHT * BL), ("Vt2T", NHT * BL), ("idh", 128),
    ("colm", BL * BL), ("cm16", BL * 16),
])
WQ, WQTOT = _layout([("qeT", NKT * BL * LQ)])
# GRU weight matrices in fp8 (inputs rq1/ct are small weighted averages;
# quantization noise lands ~5e-4 on the final softmax)
WB, WBTOT = _layout([("wihT", NKT * G), ("whhT", NKT * G)])
W32, W32TOT = _layout([("idf", 128), ("cqb", NHT), ("wb", NHT)])

_CACHED_NC = None


def _build():
    nc = bacc.Bacc("TRN2", target_bir_lowering=False, debug=False, num_devices=NC)

    peC = nc.dram_tensor("peC", [NPC, BL, 128, NKT, 512], F16, kind="ExternalInput").ap()
    peN8 = nc.dram_tensor("peN8", [NPR, 128, BL, 2, 512], F8, kind="ExternalInput").ap()
    wpA = nc.dram_tensor("wpA", [128, WATOT], F16, kind="ExternalInput").ap()
    wpQ = nc.dram_tensor("wpQ", [128, WQTOT], F16, kind="ExternalInput").ap()
    wpN = nc.dram_tensor("wpN", [LQ, BL * D], F16, kind="ExternalInput").ap()
    wpG = nc.dram_tensor("wpG", [BL, 2 * G], F16, kind="ExternalInput").ap()
    wpB = nc.dram_tensor("wpB", [128, WBTOT], F8, kind="ExternalInput").ap()
    wp32 = nc.dram_tensor("wp32", [128, W32TOT], F32, kind="ExternalInput").ap()
    out = nc.dram_tensor("out", [2, BL, LP], F32, kind="ExternalOutput").ap()

    with tile.TileContext(nc) as tc:
        sing = tc.alloc_tile_pool(name="sing", bufs=1)

        def _single(shape, dtype, name):
            return sing.tile(shape, dtype, name=name, tag=name)

        chunkp = tc.alloc_tile_pool(name="chunk", bufs=3)
        t2p = tc.alloc_tile_pool(name="t2", bufs=2)
        t2bp = tc.alloc_tile_pool(name="t2b", bufs=2)
        pnp = tc.alloc_tile_pool(name="pn", bufs=3)
        rowp = tc.alloc_tile_pool(name="rows", bufs=1)
        apb = tc.alloc_tile_pool(name="apb", bufs=2)
        wmp = tc.alloc_tile_pool(name="wm", bufs=2)
        # PSUM budget: ppp 3 banks + rowps 4 + trp 1 = 8
        ppp = tc.alloc_tile_pool(name="ppp", bufs=3, space="PSUM")
        rowps = tc.alloc_tile_pool(name="rowps", bufs=4, space="PSUM")
        trp = tc.alloc_tile_pool(name="trp", bufs=1, space="PSUM")

        # ---- packed weights, hot-first (Q phase gates the pipeline) ----
        wpA_s = _single([128, WATOT], F16, "wpA_s")
        nc.sync.dma_start(wpA_s, wpA)
        wpQ_s = _single([128, WQTOT], F16, "wpQ_s")
        nc.sync.dma_start(wpQ_s, wpQ)
        wp32_s = _single([128, W32TOT], F32, "wp32_s")
        nc.sync.dma_start(wp32_s, wp32)
        wpN_s = _single([LQ, BL * D], F16, "wpN_s")   # qeN, needed at q3
        wpG_s = _single([BL, 2 * G], F16, "wpG_s")    # GRU biases, needed late
        wpB_s = _single([128, WBTOT], F8, "wpB_s")    # GRU weights fp8

        def sA(name):
            o, ln = WA[name]
            return wpA_s[:, o:o + ln]

        WQvT_s = sA("WQvT").rearrange("p (kt h) -> p kt h", kt=NHT)
        WQuT_s = sA("WQuT").rearrange("p (kt h) -> p kt h", kt=NKT)
        WPhT_s = sA("WPhT").rearrange("p (kt h) -> p kt h", kt=NKT)
        WahT_s = sA("WahT").rearrange("p (kt h) -> p kt h", kt=NKT)
        VQrT_s = sA("VQrT").rearrange("p (ht o) -> p ht o", ht=NHT)
        Vt1T_s = sA("Vt1T").rearrange("p (ht b) -> p ht b", ht=NHT)
        Vt2T_s = sA("Vt2T").rearrange("p (ht b) -> p ht b", ht=NHT)
        idh_s = sA("idh")
        colm_s = sA("colm").rearrange("p (b c) -> p b c", b=BL)
        qeT_s = wpQ_s[:, WQ["qeT"][0]:WQ["qeT"][0] + NKT * BL * LQ].rearrange(
            "p (kt bq) -> p kt bq", kt=NKT)
        qeN_s = wpN_s[:, :]
        # fp8 GRU weights viewed as (pair, sub) for DoubleRow matmuls
        wihT_s = wpB_s[:, WB["wihT"][0]:WB["wihT"][0] + NKT * G].rearrange(
            "p (pr sub g) -> p pr sub g", pr=NKT // 2, sub=2)
        whhT_s = wpB_s[:, WB["whhT"][0]:WB["whhT"][0] + NKT * G].rearrange(
            "p (pr sub g) -> p pr sub g", pr=NKT // 2, sub=2)
        bgru_s = wpG_s.rearrange("b (two g) -> b two g", two=2)
        idf_s = wp32_s[:, W32["idf"][0]:W32["idf"][0] + 128]
        cqb_s = wp32_s[:, W32["cqb"][0]:W32["cqb"][0] + NHT]
        wb_s = wp32_s[:, W32["wb"][0]:W32["wb"][0] + NHT].rearrange(
            "p (ht o) -> p ht o", ht=NHT)

        # peN8: streamed per-pair fp8 tiles (4 in flight), fetched as P1
        # progresses so peC keeps HBM bandwidth early on
        pn_tiles = {}

        def fetch_pair(pr):
            t = pnp.tile([128, BL, 2, 512], F8, tag="pn", name=f"pn{pr}")
            nc.gpsimd.dma_start(t, peN8[pr])
            pn_tiles[pr] = t

        # persistent activations
        ppr_s = _single([128, NHT, BL, NPC, 512], F16, "ppr_s")  # raw passP
        biasP_s = _single([128, 2, NHT, BL], F32, "biasP_s")
        rq1_s = _single([BL, D], F32, "rq1_s")
        cthat_s = _single([BL, D], F32, "cthat_s")  # unnormalized ct
        rq2_s = _single([BL, D], F32, "rq2_s")
        rq1T_s = [_single([128, BL], F16, f"rq1T{k}") for k in range(NKT)]
        ctT_s = [_single([128, BL], F16, f"ctT{k}") for k in range(NKT)]
        rq2T_s = [_single([128, BL], F16, f"rq2T{k}") for k in range(NKT)]
        # fp8 copies (pair-packed) for the DR GRU matmuls; M padded to 16
        # because dual-fp8 ldweights needs the k-subtile stride % 16 == 0
        rq1T8_s = [_single([128, 2, 16], F8, f"rq1T8{k}") for k in range(NKT // 2)]
        ctT8_s = [_single([128, 2, 16], F8, f"ctT8{k}") for k in range(NKT // 2)]
        # widened eye mask [b, 16]: 1 at m==b (m<8), 0 elsewhere -- keeps the
        # DR lhsT slices 16-byte aligned (dual-fp8 ldweights requirement)
        colm8_s = _single([128, BL, 16], F8, "colm8_s")
        cm16_s = sA("cm16").rearrange("p (b m) -> p b m", b=BL)
        z1_s = _single([BL, 1], F32, "z1_s")
        rz1_s = _single([BL, 1], F32, "rz1_s")

        # masked per-batch stationary operands (column b kept, rest zero)
        nc.vector.tensor_copy(colm8_s, cm16_s)
        vt1m, vt2m = [], []
        for b in range(BL):
            m1 = _single([128, NHT, BL], F16, f"vt1m{b}")
            nc.vector.memset(m1, 0.0)
            nc.vector.tensor_copy(m1[:, :, b:b + 1], Vt1T_s[:, :, b:b + 1])
            vt1m.append(m1)
            m2 = _single([128, NHT, BL], F16, f"vt2m{b}")
            nc.vector.memset(m2, 0.0)
            nc.vector.tensor_copy(m2[:, :, b:b + 1], Vt2T_s[:, :, b:b + 1])
            vt2m.append(m2)

        def bcast_dim(ap, axis, size):
            """Insert a stride-0 (broadcast) free dim at position axis."""
            entries = list(ap.ap)
            entries.insert(axis, [0, size])
            return bass.AP(tensor=ap.tensor, offset=ap.offset, ap=entries)

        def rows_to_colsT(src_rows, dst_tiles, dst8=None, scale8=1.0):
            """src [8, 512] f32 -> four [128, 8] f16 tiles (+ fp8 pairs)."""
            for kt in range(NKT):
                ps_t = ppp.tile([128, BL], F32, tag="acc", name="ps_t")
                nc.tensor.transpose(ps_t, src_rows[:, kt * 128:(kt + 1) * 128],
                                    idf_s[:BL, :BL])
                nc.vector.tensor_copy(dst_tiles[kt], ps_t)
                if dst8 is not None:
                    nc.vector.tensor_scalar(dst8[kt // 2][:, kt % 2, 0:BL], ps_t,
                                            scale8, None, op0=OP.mult)

        def wah_bias(rqT, st):
            """biasP[:, st, ht, :] = WahT.T @ rqT + (WPh_b + Wah_b)."""
            for ht in range(NHT):
                ps_w = ppp.tile([128, BL], F32, tag="acc", name="ps_w")
                for kt in range(NKT):
                    nc.tensor.matmul(ps_w, lhsT=WahT_s[:, kt, ht * 128:(ht + 1) * 128],
                                     rhs=rqT[kt], start=kt == 0, stop=kt == NKT - 1)
                nc.vector.tensor_scalar(biasP_s[:, st, ht, :], ps_w, wb_s[:, ht, :],
                                        None, op0=OP.add)

        # ========== P1: passP + tanh + sP1 + online exp; ct via fp8 DR ====
        w1_s = rowp.tile([BL, LP], F16, tag="bigrow", name="w1_s")  # exp(sP1)
        zpart = _single([BL, NPC], F32, "zpart")
        ps_ct = rowps.tile([BL, D], F32, tag="row", name="ps_ct")
        ps_sp1 = {}
        t2saved = {}

        # peC arrives as b-pairs: one DMA trigger per 2 batches (trigger
        # instructions cost ~700ns on the issuing queue), alternating between
        # the sync and gpsimd queues
        peCr = peC.rearrange("pc b p kt d -> pc p b kt d")
        pe_tiles = {}

        def em_a_dma(pc, bp):
            petc2 = chunkp.tile([128, 2, NKT, 512], F16, tag="pe", name="petc2")
            eng = nc.sync if bp % 2 == 0 else nc.gpsimd
            eng.dma_start(petc2, peCr[pc, :, 2 * bp:2 * bp + 2])
            pe_tiles[(pc, bp)] = petc2

        def em_a(pc, b):
            if b % 2 == 0:
                em_a_dma(pc, b // 2)
            petc2 = pe_tiles[(pc, b // 2)] if b % 2 == 0 else \
                pe_tiles.pop((pc, b // 2))
            ps_pps = [ppp.tile([128, 512], F32, tag="acc", name=f"ps_pp{ht}")
                      for ht in range(NHT)]
            for kt in range(NKT):
                for ht in range(NHT):
                    nc.tensor.matmul(ps_pps[ht],
                                     lhsT=WPhT_s[:, kt, ht * 128:(ht + 1) * 128],
                                     rhs=petc2[:, b % 2, kt, :],
                                     start=kt == 0, stop=kt == NKT - 1)
            for ht in range(NHT):
                nc.vector.tensor_copy(ppr_s[:, ht, b, pc, :], ps_pps[ht])

        def em_btanh(pc, b):
            t2 = t2p.tile([128, NHT, 512], F16, tag="t2", name="t2a")
            for ht in range(NHT):
                nc.scalar.activation(t2[:, ht, :], ppr_s[:, ht, b, pc, :],
                                     AF.Tanh, bias=biasP_s[:, 0, ht, b:b + 1],
                                     scale=1.0)
            t2saved[(pc, b)] = t2

        def em_bsp(pc, b):
            t2 = t2saved.pop((pc, b))
            for ht in range(NHT):
                nc.tensor.matmul(ps_sp1[pc], lhsT=vt2m[b][:, ht, :],
                                 rhs=t2[:, ht, :],
                                 start=(b == 0 and ht == 0),
                                 stop=(b == BL - 1 and ht == NHT - 1))

        def stage_ab(apc, bpc, prefetch=()):
            for pr in prefetch:
                fetch_pair(pr)
            if bpc is not None:
                ps_sp1[bpc] = rowps.tile([BL, 512], F32, tag="row",
                                         name=f"ps_sp1_{bpc}")
            prev = None
            for b in range(BL):
                if apc is not None:
                    em_a(apc, b)
                if bpc is not None:
                    em_btanh(bpc, b)
                    if prev is not None:
                        em_bsp(bpc, prev)
                    prev = b
            if bpc is not None:
                em_bsp(bpc, prev)
                nc.scalar.activation(w1_s[:, bpc * 512:(bpc + 1) * 512],
                                     ps_sp1[bpc], AF.Exp,
                                     accum_out=zpart[:, bpc:bpc + 1])

        def stage_c(pc):
            """ct += w1[:, pc chunk] @ passEnc chunk, via fp8 DoubleRow."""
            for j in range(2):
                pr = pc * 2 + j
                wm8 = wmp.tile([128, 2, BL, 16], F8, tag="wm", name="wm8")
                for sub in range(2):
                    pt = pr * 2 + sub
                    ps_wt = trp.tile([128, BL], F16, tag="tr", name="ps_wt")
                    nc.tensor.transpose(
                        ps_wt, w1_s[:, pt * 128:(pt + 1) * 128], idh_s[:BL, :BL])
                    nc.vector.tensor_mul(wm8[:, sub, :, :],
                                         bcast_dim(ps_wt[:, :], 2, 16),
                                         colm8_s[:, :, :])
                pn = pn_tiles.pop(pr)
                for b in range(BL):
                    nc.tensor.matmul(
                        ps_ct, lhsT=wm8[:, :, b, 0:BL], rhs=pn[:, b, :, :],
                        start=(pr == 0 and b == 0),
                        stop=(pr == NPR - 1 and b == BL - 1),
                        perf_mode=DR)

        gi_s = _single([BL, G], F16, "gi_s")

        # ---- Q phase, split into 4 chunks interleaved with A(0) ----
        tqT_s = _single([128, NHT, BL * LQ], F16, "tqT_s")
        cb_s = _single([128, NHT], F32, "cb_s")
        esq = _single([BL, LQ], F32, "esq")
        zq = _single([BL, 1], F32, "zq")
        rzq = _single([BL, 1], F32, "rzq")
        a_s = _single([BL, LQ], F16, "a_s")
        atm_s = _single([LQ, BL, BL], F16, "atm_s")

        def q1():
            ps_qv = trp.tile([128, NHT], F32, tag="tr", name="ps_qv")
            for ht in range(NHT):
                for kt in range(NHT):
                    nc.tensor.matmul(ps_qv[:, ht:ht + 1],
                                     lhsT=WQvT_s[:, kt, ht * 128:(ht + 1) * 128],
                                     rhs=VQrT_s[:, kt, :], start=kt == 0,
                                     stop=kt == NHT - 1)
            nc.vector.tensor_add(cb_s, ps_qv, cqb_s)
            for ht in range(NHT):
                ps_tq = ppp.tile([128, 512], F32, tag="acc", name="ps_tq")
                for kt in range(NKT):
                    nc.tensor.matmul(ps_tq, lhsT=WQuT_s[:, kt, ht * 128:(ht + 1) * 128],
                                     rhs=qeT_s[:, kt, :], start=kt == 0,
                                     stop=kt == NKT - 1)
                nc.scalar.activation(tqT_s[:, ht, :], ps_tq, AF.Tanh,
                                     bias=cb_s[:, ht:ht + 1], scale=1.0)

        def q2():
            ps_sq = rowps.tile([BL, LQ], F32, tag="row", name="ps_sq")
            for b in range(BL):
                for ht in range(NHT):
                    nc.tensor.matmul(ps_sq, lhsT=vt1m[b][:, ht, :],
                                     rhs=tqT_s[:, ht, b * LQ:(b + 1) * LQ],
                                     start=(b == 0 and ht == 0),
                                     stop=(b == BL - 1 and ht == NHT - 1))
            nc.scalar.activation(esq, ps_sq, AF.Exp, accum_out=zq)
            nc.vector.reciprocal(rzq, zq)
            nc.vector.tensor_scalar(a_s, esq, rzq, None, op0=OP.mult)

        def q3():
            ps_at = trp.tile([LQ, BL], F16, tag="tr", name="ps_at")
            nc.tensor.transpose(ps_at, a_s, idh_s[:BL, :BL])
            nc.vector.tensor_mul(atm_s,
                                 bcast_dim(ps_at[:, :], 1, BL),
                                 colm_s[0:LQ, :, :])
            ps_rq = rowps.tile([BL, D], F32, tag="row", name="ps_rq")
            for b in range(BL):
                nc.tensor.matmul(ps_rq, lhsT=atm_s[:, b, :],
                                 rhs=qeN_s[:, b * D:(b + 1) * D],
                                 start=b == 0, stop=b == BL - 1)
            nc.vector.tensor_copy(rq1_s, ps_rq)

        def q4():
            rows_to_colsT(rq1_s, rq1T_s, rq1T8_s)
            wah_bias(rq1T_s, 0)

        def emit_gi():
            # gi = rq1 @ wih.T + bih, fp8 DoubleRow; emitted mid-P1 (after
            # the wpB/wpG DMAs are in flight, long before the GRU needs it)
            for nch in range(G // 512):
                ps_gi = rowps.tile([BL, 512], F32, tag="row", name="ps_gi")
                for pr in range(NKT // 2):
                    nc.tensor.matmul(ps_gi, lhsT=rq1T8_s[pr][:, :, 0:BL],
                                     rhs=wihT_s[:, pr, :, nch * 512:(nch + 1) * 512],
                                     start=pr == 0, stop=pr == NKT // 2 - 1,
                                     perf_mode=DR)
                nc.vector.scalar_tensor_tensor(
                    gi_s[:, nch * 512:(nch + 1) * 512], ps_gi, 1.0 / 16.0,
                    bgru_s[:, 0, nch * 512:(nch + 1) * 512],
                    op0=OP.mult, op1=OP.add)

        # ---- emit: Q interleaved with A(0), then pipelined P1 ----
        q1()
        em_a(0, 0); em_a(0, 1)
        nc.sync.dma_start(wpN_s, wpN)  # qeN for q3
        q2()
        em_a(0, 2); em_a(0, 3)
        q3()
        em_a(0, 4); em_a(0, 5)
        q4()
        em_a(0, 6); em_a(0, 7)

        stage_ab(1, 0)
        stage_ab(2, 1)
        nc.gpsimd.dma_start(wpB_s, wpB)
        nc.gpsimd.dma_start(wpG_s, wpG)
        fetch_pair(0); fetch_pair(1); fetch_pair(2)
        emit_gi()
        stage_c(0)
        fetch_pair(3); fetch_pair(4)
        stage_ab(3, 2)
        stage_c(1)
        fetch_pair(5); fetch_pair(6)
        stage_c(2)
        fetch_pair(7)
        stage_ab(None, 3)
        # 1/Z for step 1: ready while C3 matmuls run
        nc.vector.reduce_sum(z1_s, zpart, axis=mybir.AxisListType.X)
        nc.vector.reciprocal(rz1_s, z1_s)
        stage_c(3)

        # ================= GRU cell =================
        # cthat = raw PSUM sum (unnormalized); matmul is linear so
        # gh_raw = cthat @ whh.T, then gh = gh_raw * rz1 + b_hh
        # fp8 ct copy pre-scaled by 1/16 (raw sums ~±220, fp8e4 max 240);
        # the x16 host-side weight scale cancels it in ps_g. The PSUM->SBUF
        # copy is chunked per kt so transposes overlap it.
        for kt in range(NKT):
            ks = slice(kt * 128, (kt + 1) * 128)
            nc.vector.tensor_copy(cthat_s[:, ks], ps_ct[:, ks])
            ps_t = ppp.tile([128, BL], F32, tag="acc", name="ps_t")
            nc.tensor.transpose(ps_t, cthat_s[:, ks], idf_s[:BL, :BL])
            nc.vector.tensor_copy(ctT_s[kt], ps_t)
            nc.vector.tensor_scalar(ctT8_s[kt // 2][:, kt % 2, 0:BL], ps_t,
                                    1.0 / 16.0, None, op0=OP.mult)
        gh_s = _single([BL, G], F16, "gh_s")
        for nch in range(G // 512):
            ps_g = rowps.tile([BL, 512], F32, tag="row", name="ps_g")
            for pr in range(NKT // 2):
                nc.tensor.matmul(ps_g, lhsT=ctT8_s[pr][:, :, 0:BL],
                                 rhs=whhT_s[:, pr, :, nch * 512:(nch + 1) * 512],
                                 start=pr == 0, stop=pr == NKT // 2 - 1,
                                 perf_mode=DR)
            nc.vector.scalar_tensor_tensor(
                gh_s[:, nch * 512:(nch + 1) * 512], ps_g, rz1_s,
                bgru_s[:, 1, nch * 512:(nch + 1) * 512],
                op0=OP.mult, op1=OP.add)
        rzin_s = _single([BL, 2 * D], F16, "rzin_s")
        nc.vector.tensor_add(rzin_s, gi_s[:, 0:2 * D], gh_s[:, 0:2 * D])
        rz_s = _single([BL, 2 * D], F16, "rz_s")
        nc.scalar.activation(rz_s, rzin_s, AF.Sigmoid)
        nin_s = _single([BL, D], F16, "nin_s")
        nc.vector.tensor_mul(nin_s, rz_s[:, 0:D], gh_s[:, 2 * D:3 * D])
        nin2_s = _single([BL, D], F16, "nin2_s")
        nc.vector.tensor_add(nin2_s, nin_s, gi_s[:, 2 * D:3 * D])
        n_s = _single([BL, D], F16, "n_s")
        nc.scalar.activation(n_s, nin2_s, AF.Tanh)
        # h' = n + z*(ct - n); ct = cthat/Z fused into the subtract
        d1_s = _single([BL, D], F16, "d1_s")
        nc.vector.scalar_tensor_tensor(d1_s, cthat_s, rz1_s, n_s,
                                       op0=OP.mult, op1=OP.subtract)
        nc.vector.tensor_mul(d1_s, d1_s, rz_s[:, D:2 * D])
        nc.vector.tensor_add(rq2_s, n_s, d1_s)

        rows_to_colsT(rq2_s, rq2T_s)
        wah_bias(rq2T_s, 1)

        # aP1 normalize + store: off the critical path, runs during P2
        for pc in range(NPC):
            ap1c = apb.tile([BL, 512], F32, tag="ap", name="ap1c")
            nc.vector.tensor_scalar(ap1c, w1_s[:, pc * 512:(pc + 1) * 512],
                                    rz1_s, None, op0=OP.mult)
            nc.gpsimd.dma_start(out=out[0, :, pc * 512:(pc + 1) * 512], in_=ap1c)

        # ========== P2: tanh + sP2 (passP reused), raw exp ==========
        ps_sp2 = [rowps.tile([BL, 512], F32, tag="row", name=f"ps_sp2_{pc}")
                  for pc in range(NPC)]
        w2_s = rowp.tile([BL, LP], F16, tag="bigrow", name="w2_s")
        zp2 = _single([BL, NPC], F32, "zp2")
        for b in range(BL):
            t2b = t2bp.tile([128, NHT, LP], F16, tag="t2b", name="t2b")
            for ht in range(NHT):
                nc.scalar.activation(t2b[:, ht, :], ppr_s[:, ht, b, :, :], AF.Tanh,
                                     bias=biasP_s[:, 1, ht, b:b + 1], scale=1.0)
            for pc in range(NPC):
                for ht in range(NHT):
                    nc.tensor.matmul(ps_sp2[pc],
                                     lhsT=vt2m[b][:, ht, :],
                                     rhs=t2b[:, ht, pc * 512:(pc + 1) * 512],
                                     start=(b == 0 and ht == 0),
                                     stop=(b == BL - 1 and ht == NHT - 1))
                if b == BL - 1:
                    # per-pc group just stopped: exp immediately, pipelined
                    # against the remaining pc groups' matmuls
                    nc.scalar.activation(w2_s[:, pc * 512:(pc + 1) * 512],
                                         ps_sp2[pc], AF.Exp,
                                         accum_out=zp2[:, pc:pc + 1])
        z2_s = _single([BL, 1], F32, "z2_s")
        nc.vector.reduce_sum(z2_s, zp2, axis=mybir.AxisListType.X)
        rz2_s = _single([BL, 1], F32, "rz2_s")
        nc.vector.reciprocal(rz2_s, z2_s)
        out_eng = [nc.sync, nc.gpsimd, nc.scalar, nc.sync]
        for pc in range(NPC):
            ap2c = apb.tile([BL, 512], F32, tag="ap", name="ap2c")
            nc.vector.tensor_scalar(ap2c, w2_s[:, pc * 512:(pc + 1) * 512],
                                    rz2_s, None, op0=OP.mult)
            out_eng[pc].dma_start(out=out[1, :, pc * 512:(pc + 1) * 512],
                                  in_=ap2c)

        trp.release()
        rowps.release()
        ppp.release()
        wmp.release()
        apb.release()
        rowp.release()
        pnp.release()
        t2bp.release()
        t2p.release()
        chunkp.release()
        sing.release()

    nc.compile()
    return nc


def _get_nc():
    global _CACHED_NC
    if _CACHED_NC is None:
        _CACHED_NC = _build()
    return _CACHED_NC


def _tiles(mat, nkt):  # [nkt*128, X] -> [128, nkt*X]
    x = mat.shape[1]
    return np.ascontiguousarray(
        mat.reshape(nkt, 128, x).transpose(1, 0, 2).reshape(128, nkt * x))


def _packA(f, Vt1, Vt2):
    wp = np.zeros((128, WATOT), dtype=np.float16)

    def put(name, arr):
        o, ln = WA[name]
        assert arr.shape[1] == ln, (name, arr.shape, ln)
        wp[:arr.shape[0], o:o + ln] = arr

    put("WQvT", _tiles(f["WQv_W"].T.astype(np.float16), NHT))
    put("WQuT", _tiles(f["WQu_W"].T.astype(np.float16), NKT))
    put("WPhT", _tiles(f["WPh_W"].T.astype(np.float16), NKT))
    put("WahT", _tiles(f["Wah_W"].T.astype(np.float16), NKT))
    put("VQrT", _tiles(f["VQr"].reshape(1, H).T.astype(np.float16), NHT))
    put("Vt1T", _tiles(Vt1.astype(np.float16), NHT))
    put("Vt2T", _tiles(Vt2.astype(np.float16), NHT))
    put("idh", np.eye(128, dtype=np.float16))
    put("colm", np.broadcast_to(np.eye(BL, dtype=np.float16).reshape(1, BL * BL),
                                (128, BL * BL)))
    cm16 = np.hstack([np.eye(BL, dtype=np.float16),
                      np.zeros((BL, 16 - BL), dtype=np.float16)])
    put("cm16", np.broadcast_to(cm16.reshape(1, BL * 16), (128, BL * 16)))
    return wp


def _packG(f):
    return np.stack([np.broadcast_to(f["gru_bih"], (BL, G)),
                     np.broadcast_to(f["gru_bhh"], (BL, G))],
                    axis=1).astype(np.float16).reshape(BL, 2 * G)


def _packQ(qe):
    wp = np.zeros((128, WQTOT), dtype=np.float16)
    o, ln = WQ["qeT"]
    qeT = np.ascontiguousarray(qe.transpose(2, 1, 0)).astype(np.float16)
    wp[:, o:o + ln] = _tiles(qeT.reshape(D, BL * LQ), NKT)
    return wp


def _packB(f):
    # x16 lifts the ~N(0, 0.05^2) weights out of fp8's subnormal range;
    # compensated on-chip (gi: x1/16 in the bias add; gh: cancels the
    # ct fp8 copy's 1/16 pre-scale)
    wp = np.zeros((128, WBTOT), dtype=np.float32)
    o, ln = WB["wihT"]
    wp[:, o:o + ln] = _tiles(f["gru_wih"].T.astype(np.float32) * 16.0, NKT)
    o, ln = WB["whhT"]
    wp[:, o:o + ln] = _tiles(f["gru_whh"].T.astype(np.float32) * 16.0, NKT)
    return _fp8(wp)


def _pack32(f):
    wp = np.zeros((128, W32TOT), dtype=np.float32)
    o, ln = W32["idf"]
    wp[:, o:o + ln] = np.eye(128, dtype=np.float32)
    o, ln = W32["cqb"]
    wp[:, o:o + ln] = (f["WQu_b"] + f["WQv_b"]).astype(np.float32).reshape(NHT, 128).T
    o, ln = W32["wb"]
    wp[:, o:o + ln] = (f["WPh_b"] + f["Wah_b"]).astype(np.float32).reshape(NHT, 128).T
    return wp


def _fp8(x):
    import ml_dtypes
    return np.ascontiguousarray(x).astype(ml_dtypes.float8_e4m3).view(np.uint8)


def make_in_maps(f):
    passEnc, quesEnc = f["passEnc"], f["quesEnc"]
    wp32 = _pack32(f)
    wpB = _packB(f)
    in_maps = []
    for i in range(NC):
        s = slice(i * BL, (i + 1) * BL)
        pe = passEnc[:, s, :]
        qe = quesEnc[:, s, :]
        wpA = _packA(f, f["Vt1"][s, :, 0].T, f["Vt2"][s, :, 0].T)
        wpQ_ = _packQ(qe)
        peC = np.ascontiguousarray(
            pe.astype(np.float16).reshape(NPC, 512, BL, NKT, 128).transpose(
                0, 2, 4, 3, 1))
        # peN8 [pair, part, b, sub, d]: global p = pair*256 + sub*128 + part
        peN8 = _fp8(pe.reshape(NPR, 2, 128, BL, D).transpose(0, 2, 3, 1, 4))
        in_maps.append({
            "peC": peC,
            "peN8": peN8,
            "wpA": wpA, "wpQ": wpQ_, "wpB": wpB, "wp32": wp32,
            "wpN": qe.astype(np.float16).reshape(LQ, BL * D),
            "wpG": _packG(f),
        })
    return in_maps


def kernel(**inputs):
    f = {k: np.asarray(v) for k, v in inputs.items()}
    in_maps = make_in_maps(f)
    nc = _get_nc()
    res = run_bass_kernel_spmd(nc, in_maps, core_ids=list(range(NC)))
    aP1 = np.concatenate([res.results[i]["out"][0] for i in range(NC)], axis=0)
    aP2 = np.concatenate([res.results[i]["out"][1] for i in range(NC)], axis=0)
    return (aP1.astype(np.float32), aP2.astype(np.float32))
